# revision 4
# baseline (speedup 1.0000x reference)
"""Cached Gemma attention decode step on 8 Trainium2 NeuronCores.

Sharding: projections are D-sharded (each core owns a 256-wide slice of the
model dim for Wq/Wk/Wv and of the output dim for Wo); a ReduceScatter combines
partial q/k/v and scatters batches; attention over the KV cache is
batch-sharded (4 batches per core); an AllGather collects attention vectors
for the D-sharded output projection. The kernel returns
(attention_output, new_cache) exactly like the reference.
"""

import os
import sys

sys.path.insert(0, "/opt/trn_rl_repo")

import numpy as np

B, T, D = 32, 1, 2048
S = 4096
NQ, NKV, H = 8, 1, 256
CIDX = 2048
MAX_WAVELENGTH = 10000.0
NCORES = 8
BPC = B // NCORES          # batches per core
DSL = D // NCORES          # model-dim slice per core
SC = CIDX                  # cached positions attended (0..CIDX-1) + new token
NCH = SC // 512            # 512-wide s-chunks per batch

_BUILT = {}


def _rope_tables():
    # float32 throughout, matching the reference's jax-on-cpu numerics
    half = H // 2
    fe = ((2.0 / H) * np.arange(half, dtype=np.float32)).astype(np.float32)
    ts = np.power(np.float32(MAX_WAVELENGTH), fe).astype(np.float32)
    rad = (np.float32(CIDX) / ts).astype(np.float32)
    return np.cos(rad).astype(np.float32), np.sin(rad).astype(np.float32)


def _build(tf32=True):
    import concourse.bacc as bacc
    import concourse.tile as tile
    import concourse.mybir as mybir
    from concourse.masks import make_identity

    f32 = mybir.dt.float32
    rdt = mybir.dt.float32r if tf32 else f32
    AX = mybir.AxisListType
    OP = mybir.AluOpType
    AF = mybir.ActivationFunctionType

    nc = bacc.Bacc("TRN2", target_bir_lowering=False, debug=False, num_devices=NCORES)

    xt = nc.dram_tensor("xt", [DSL, B], f32, kind="ExternalInput")
    w1 = nc.dram_tensor("w1", [DSL, NQ * H + 2 * H], f32, kind="ExternalInput")
    kc = nc.dram_tensor("kc", [BPC, SC, H], f32, kind="ExternalInput")
    vc = nc.dram_tensor("vc", [BPC, SC, H], f32, kind="ExternalInput")
    wo = nc.dram_tensor("wo", [NQ * H, DSL], f32, kind="ExternalInput")
    cosb = nc.dram_tensor("cosb", [BPC, H // 2], f32, kind="ExternalInput")
    sinb = nc.dram_tensor("sinb", [BPC, H // 2], f32, kind="ExternalInput")

    out_d = nc.dram_tensor("out_d", [B, DSL], f32, kind="ExternalOutput")
    knew = nc.dram_tensor("knew", [BPC, H], f32, kind="ExternalOutput")
    vnew = nc.dram_tensor("vnew", [BPC, H], f32, kind="ExternalOutput")

    P1W = NQ * H + 2 * H  # 2560

    with tile.TileContext(nc) as tc:
        with (
            tc.tile_pool(name="const", bufs=1) as const,
            tc.tile_pool(name="kst", bufs=4) as kstp,
            tc.tile_pool(name="vst", bufs=3) as vstp,
            tc.tile_pool(name="ktc", bufs=3) as ktp,
            tc.tile_pool(name="vrf", bufs=2) as vrfp,
            tc.tile_pool(name="prb", bufs=3) as prbp,
            tc.tile_pool(name="sml", bufs=4) as smlp,
            tc.tile_pool(name="ptp", bufs=3) as ptpp,
            tc.tile_pool(name="att", bufs=4) as attp,
            tc.tile_pool(name="tpp", bufs=4, space="PSUM") as tpp,
            tc.tile_pool(name="mmp", bufs=2, space="PSUM") as mmp,
            tc.tile_pool(name="avp", bufs=2, space="PSUM") as avpp,
            tc.tile_pool(name="dram", bufs=1, space="DRAM") as dram,
        ):
            ident = const.tile([128, 128], f32)
            make_identity(nc, ident)

            # ---- phase 1: d-sharded projections + ReduceScatter ----
            xt_t = const.tile([128, 2, B], f32)
            nc.sync.dma_start(out=xt_t, in_=xt.ap().rearrange("(c p) b -> p c b", p=128))
            w1_t = const.tile([128, 2, P1W], f32)
            nc.sync.dma_start(out=w1_t, in_=w1.ap().rearrange("(c p) m -> p c m", p=128))

            p1sb = const.tile([B, P1W], f32)
            for j in range(P1W // 512):
                p1 = mmp.tile([B, 512], f32, tag="mm")
                for c in range(2):
                    nc.tensor.matmul(
                        p1,
                        xt_t[:, c, :],
                        w1_t[:, c, 512 * j : 512 * (j + 1)],
                        start=(c == 0),
                        stop=(c == 1),
                    )
                nc.any.tensor_copy(p1sb[:, 512 * j : 512 * (j + 1)], p1)

            p1stage = dram.tile([B, P1W], f32)
            rsout = dram.tile([BPC, P1W], f32)
            nc.gpsimd.dma_start(out=p1stage[:, :], in_=p1sb)
            nc.gpsimd.collective_compute(
                "ReduceScatter",
                OP.add,
                replica_groups=[list(range(NCORES))],
                ins=[p1stage.opt()],
                outs=[rsout.opt()],
            )
            qkv = const.tile([BPC, P1W], f32)
            nc.sync.dma_start(out=qkv, in_=rsout[:, :])

            # ---- rope on q (8 heads) and k ----
            cost = const.tile([BPC, H // 2], f32)
            sint = const.tile([BPC, H // 2], f32)
            nc.sync.dma_start(out=cost, in_=cosb[:, :])
            nc.sync.dma_start(out=sint, in_=sinb[:, :])

            qrot = const.tile([BPC, NQ * H], f32)
            krot = const.tile([BPC, H], f32)
            t1 = const.tile([BPC, H // 2], f32)
            t2 = const.tile([BPC, H // 2], f32)
            hh = H // 2

            def rope(dst_ap, x1, x2):
                # dst even lanes = x1*cos - x2*sin ; odd = x2*cos + x1*sin
                ev = dst_ap.rearrange("p (h two) -> p h two", two=2)[:, :, 0]
                od = dst_ap.rearrange("p (h two) -> p h two", two=2)[:, :, 1]
                nc.vector.tensor_mul(t1, x1, cost)
                nc.vector.tensor_mul(t2, x2, sint)
                nc.vector.tensor_sub(ev, t1, t2)
                nc.vector.tensor_mul(t1, x2, cost)
                nc.vector.tensor_mul(t2, x1, sint)
                nc.vector.tensor_add(od, t1, t2)

            for n in range(NQ):
                o = n * H
                rope(qrot[:, o : o + H], qkv[:, o : o + hh], qkv[:, o + hh : o + H])
            ko = NQ * H
            rope(krot[:, :], qkv[:, ko : ko + hh], qkv[:, ko + hh : ko + H])

            nc.gpsimd.dma_start(out=knew[:, :], in_=krot)
            nc.gpsimd.dma_start(out=vnew[:, :], in_=qkv[:, ko + H : ko + 2 * H])
            # v_new rows gathered onto partition 0 so the final AV matmul's
            # operands share a partition base
            vn0 = const.tile([1, BPC, H], f32)
            nc.gpsimd.dma_start(out=vn0, in_=qkv[:, ko + H : ko + 2 * H])
            vnr = const.tile([1, BPC, H], rdt)
            nc.vector.tensor_copy(vnr, vn0)

            # ---- qT [128, 16, BPC] and kT_new [128, 2, BPC] (contract dim on partitions) ----
            qTf = const.tile([128, 16, BPC], rdt)
            for j in range(16):
                tp = tpp.tile([128, BPC], f32, tag="tp")
                nc.tensor.transpose(
                    tp, qrot[:, 128 * j : 128 * (j + 1)], ident[0:BPC, 0:BPC]
                )
                nc.vector.tensor_copy(qTf[:, j, :], tp)
            kTn = const.tile([128, 2, BPC], rdt)
            for hc in range(2):
                tp = tpp.tile([128, BPC], f32, tag="tp")
                nc.tensor.transpose(
                    tp, krot[:, 128 * hc : 128 * (hc + 1)], ident[0:BPC, 0:BPC]
                )
                nc.vector.tensor_copy(kTn[:, hc, :], tp)

            # ---- attention, batch-sharded ----
            astage = dram.tile([BPC, NQ * H], f32)
            for b in range(BPC):
                qT_b = qTf[:, :, b].rearrange("p (n hc) -> p hc n", hc=2)
                probs = prbp.tile([NQ, SC + 1], f32, tag="probs")
                sums = smlp.tile([NQ, NCH + 1], f32, tag="sums")
                av = avpp.tile([NQ, H], f32, tag="av")

                for j in range(NCH):
                    kst = kstp.tile([128, 4, H], f32, tag="kst")
                    nc.sync.dma_start(
                        out=kst,
                        in_=kc[b, 512 * j : 512 * (j + 1), :].rearrange(
                            "(t p) h -> p t h", p=128
                        ),
                    )
                    ktc = ktp.tile([128, 2, 512], rdt, tag="ktc")
                    for i in range(4):
                        for hc in range(2):
                            tp = tpp.tile([128, 128], f32, tag="tp")
                            nc.tensor.transpose(
                                tp, kst[:, i, 128 * hc : 128 * (hc + 1)], ident
                            )
                            nc.vector.tensor_copy(
                                ktc[:, hc, 128 * i : 128 * (i + 1)], tp
                            )
                    lg = mmp.tile([NQ, 512], f32, tag="mm")
                    for hc in range(2):
                        nc.tensor.matmul(
                            lg,
                            qT_b[:, hc, :],
                            ktc[:, hc, :],
                            start=(hc == 0),
                            stop=(hc == 1),
                        )
                    nc.scalar.activation(
                        out=probs[:, 512 * j : 512 * (j + 1)],
                        in_=lg,
                        func=AF.Exp,
                        accum_out=sums[:, j : j + 1],
                    )
                    # probsT for this chunk
                    ptt = tpp.tile([128, 4, NQ], f32, tag="tp")
                    for i in range(4):
                        nc.tensor.matmul(
                            ptt[:, i, :],
                            probs[:, 512 * j + 128 * i : 512 * j + 128 * (i + 1)],
                            ident[0:NQ, 0:NQ],
                            is_transpose=True,
                            start=(i == 0),
                            stop=(i == 3),
                        )
                    pT = ptpp.tile([128, 4, NQ], rdt, tag="pT")
                    nc.vector.tensor_copy(pT, ptt)

                    vst = vstp.tile([128, 4, H], f32, tag="vst")
                    nc.scalar.dma_start(
                        out=vst,
                        in_=vc[b, 512 * j : 512 * (j + 1), :].rearrange(
                            "(t p) h -> p t h", p=128
                        ),
                    )
                    vrf = vrfp.tile([128, 4, H], rdt, tag="vrf")
                    nc.vector.tensor_copy(vrf, vst)
                    for i in range(4):
                        nc.tensor.matmul(
                            av,
                            pT[:, i, :],
                            vrf[:, i, :],
                            start=(j == 0 and i == 0),
                            stop=False,
                        )

                # the freshly-written cache row (position CIDX); tiny N=1/K=1
                # matmuls run as plain fp32 (fp32r trips an ISA check there)
                lgl = mmp.tile([NQ, 1], f32, tag="mm")
                for hc in range(2):
                    nc.tensor.matmul(
                        lgl,
                        qT_b[:, hc, :].bitcast(f32),
                        kTn[:, hc, b : b + 1].bitcast(f32),
                        start=(hc == 0),
                        stop=(hc == 1),
                    )
                nc.scalar.activation(
                    out=probs[:, SC : SC + 1],
                    in_=lgl,
                    func=AF.Exp,
                    accum_out=sums[:, NCH : NCH + 1],
                )
                # [8,1] -> [1,8] via SBUF->SBUF DMA (partition dim to free dim)
                pTl = ptpp.tile([1, NQ], f32, tag="pTl")
                nc.gpsimd.dma_start(out=pTl, in_=probs[:, SC : SC + 1])
                nc.tensor.matmul(
                    av, pTl, vnr[0:1, b, :].bitcast(f32), start=False, stop=True
                )

                stot = smlp.tile([NQ, 1], f32, tag="stot")
                rec = smlp.tile([NQ, 1], f32, tag="rec")
                nc.vector.tensor_reduce(stot, sums, axis=AX.X, op=OP.add)
                nc.vector.reciprocal(rec, stot)
                attn = attp.tile([NQ, H], f32, tag="attn")
                nc.scalar.activation(out=attn, in_=av, func=AF.Copy, bias=0.0, scale=rec)
                nc.gpsimd.dma_start(out=astage[b, :], in_=attn)

            # ---- AllGather + d-sharded output projection ----
            aall = dram.tile([B, NQ * H], f32)
            nc.gpsimd.collective_compute(
                "AllGather",
                OP.bypass,
                replica_groups=[list(range(NCORES))],
                ins=[astage.opt()],
                outs=[aall.opt()],
            )
            aal = const.tile([B, NQ * H], f32)
            nc.sync.dma_start(out=aal, in_=aall[:, :])

            wo_t = const.tile([128, 16, DSL], f32)
            nc.scalar.dma_start(
                out=wo_t, in_=wo.ap().rearrange("(j p) d -> p j d", p=128)
            )
            wo_r = const.tile([128, 16, DSL], rdt)
            nc.vector.tensor_copy(wo_r, wo_t)

            aT = const.tile([128, 16, B], rdt)
            for j in range(16):
                tp = tpp.tile([128, B], f32, tag="tp")
                nc.tensor.transpose(
                    tp, aal[:, 128 * j : 128 * (j + 1)], ident[0:B, 0:B]
                )
                nc.vector.tensor_copy(aT[:, j, :], tp)
            po = mmp.tile([B, DSL], f32, tag="mm")
            for j in range(16):
                nc.tensor.matmul(
                    po, aT[:, j, :], wo_r[:, j, :], start=(j == 0), stop=(j == 15)
                )
            osb = const.tile([B, DSL], f32)
            nc.any.tensor_copy(osb, po)
            nc.gpsimd.dma_start(out=out_d[:, :], in_=osb)

    nc.compile()
    return nc


def _numpy_reference(x, attention_mask, cache, Wq, Wk, Wv, Wo, cache_update_index):
    """Faithful numpy port of the reference, used only as a safety fallback."""
    b, t, d = x.shape
    nq, _, h = Wq.shape
    nkv = Wk.shape[0]
    g = nq // nkv
    idx = int(cache_update_index)
    positions = np.arange(t, dtype=np.float32) + np.float32(idx)

    def rope(xx, pos):
        hh = xx.shape[-1] // 2
        fe = (2.0 / xx.shape[-1]) * np.arange(hh, dtype=np.float32)
        ts = MAX_WAVELENGTH ** fe
        rad = pos[:, None] / ts[None, :]
        rad = rad[None, :, None, :]
        sin, cos = np.sin(rad), np.cos(rad)
        x1, x2 = xx[..., :hh], xx[..., hh:]
        out = np.stack([x1 * cos - x2 * sin, x2 * cos + x1 * sin], axis=-1)
        return out.reshape(xx.shape).astype(np.float32)

    query = np.einsum("btd,ndh->btnh", x, Wq).astype(np.float32)
    query = rope(query, positions)
    key_u = rope(np.einsum("btd,kdh->btkh", x, Wk).astype(np.float32), positions)
    val_u = np.einsum("btd,kdh->btkh", x, Wv).astype(np.float32)
    key = cache[:, 0].copy()
    val = cache[:, 1].copy()
    key[:, idx : idx + t] = key_u
    val[:, idx : idx + t] = val_u
    new_cache = np.stack([key, val], axis=1)

    q = (query * (1.0 / np.sqrt(h))).reshape(b, t, nkv, g, h)
    logits = np.einsum("btkgh,bskh->bkgts", q, key).astype(np.float32)
    mask = attention_mask[:, None, None, :, :]
    logits = np.where(mask, logits, -1e9)
    m = logits.max(axis=-1, keepdims=True)
    e = np.exp(logits - m)
    probs = e / e.sum(axis=-1, keepdims=True)
    attn = np.einsum("bkgts,bskh->btkgh", probs, val).reshape(b, t, nq, h)
    noatt = np.all(~attention_mask, axis=-1)[..., None, None]
    attn = np.where(noatt, 0.0, attn)
    out = np.einsum("btnh,nhd->btd", attn, Wo).astype(np.float32)
    return out, new_cache.astype(np.float32)


def _standard_case(x, attention_mask, cache, Wq, Wk, Wv, Wo, cache_update_index):
    if int(cache_update_index) != CIDX:
        return False
    if x.shape != (B, T, D) or cache.shape != (B, 2, S, NKV, H):
        return False
    expect = np.arange(S) <= CIDX
    return bool(np.all(attention_mask == expect[None, None, :]))


def kernel(x, attention_mask, cache, Wq, Wk, Wv, Wo, cache_update_index):
    x = np.asarray(x, dtype=np.float32)
    attention_mask = np.asarray(attention_mask).astype(bool)
    cache = np.asarray(cache, dtype=np.float32)
    Wq = np.asarray(Wq, dtype=np.float32)
    Wk = np.asarray(Wk, dtype=np.float32)
    Wv = np.asarray(Wv, dtype=np.float32)
    Wo = np.asarray(Wo, dtype=np.float32)

    if not _standard_case(x, attention_mask, cache, Wq, Wk, Wv, Wo, cache_update_index):
        return _numpy_reference(
            x, attention_mask, cache, Wq, Wk, Wv, Wo, cache_update_index
        )

    from concourse.bass_utils import run_bass_kernel_spmd

    tf32 = os.environ.get("KERNEL_TF32", "1") == "1"
    trace = os.environ.get("KERNEL_TRACE", "0") == "1"
    key = ("nc", tf32)
    if key not in _BUILT:
        _BUILT[key] = _build(tf32=tf32)
    nc = _BUILT[key]

    cos, sin = _rope_tables()
    cosb = np.broadcast_to(cos, (BPC, H // 2)).copy()
    sinb = np.broadcast_to(sin, (BPC, H // 2)).copy()

    x2 = x[:, 0, :]                              # [B, D]
    Wq_s = (Wq * np.float32(1.0 / np.sqrt(H))).astype(np.float32)
    in_maps = []
    for c in range(NCORES):
        dsl = slice(DSL * c, DSL * (c + 1))
        bsl = slice(BPC * c, BPC * (c + 1))
        w1 = np.empty((DSL, NQ * H + 2 * H), dtype=np.float32)
        w1[:, : NQ * H] = (
            Wq_s[:, dsl, :].transpose(1, 0, 2).reshape(DSL, NQ * H)
        )
        w1[:, NQ * H : NQ * H + H] = Wk[0][dsl, :]
        w1[:, NQ * H + H :] = Wv[0][dsl, :]
        in_maps.append(
            {
                "xt": np.ascontiguousarray(x2[:, dsl].T),
                "w1": w1,
                "kc": np.ascontiguousarray(cache[bsl, 0, :SC, 0, :]),
                "vc": np.ascontiguousarray(cache[bsl, 1, :SC, 0, :]),
                "wo": np.ascontiguousarray(Wo[:, :, dsl].reshape(NQ * H, DSL)),
                "cosb": cosb,
                "sinb": sinb,
            }
        )

    res = run_bass_kernel_spmd(nc, in_maps, list(range(NCORES)), trace=trace)
    if trace and res.exec_time_ns is not None:
        print(f"HW exec time: {res.exec_time_ns} ns")

    out = np.empty((B, T, D), dtype=np.float32)
    new_cache = cache.copy().reshape(B, 2, S, NKV, H)
    for c in range(NCORES):
        r = res.results[c]
        out[:, 0, DSL * c : DSL * (c + 1)] = r["out_d"]
        bsl = slice(BPC * c, BPC * (c + 1))
        new_cache[bsl, 0, CIDX, 0, :] = r["knew"]
        new_cache[bsl, 1, CIDX, 0, :] = r["vnew"]

    # no_attended rows (never triggered by the standard mask, kept for parity)
    noatt = np.all(~attention_mask, axis=-1)
    if noatt.any():
        out[noatt[:, 0], :] = 0.0  # pragma: no cover

    return out, new_cache
